# revision 7
# baseline (speedup 1.0000x reference)
"""AttentionPooling Trainium2 kernel, v3 ("ARCH2", d-major, transpose-free).

Math (per batch row b):
    x   = target[b] + hist[b]              # [T, D]
    h   = relu(x @ W + Wb)                 # [T, D]
    lg  = h @ q  (+ q_bias, softmax-invariant -> ignored)
    s   = softmax(lg) over T
    out = sum_t s_t * hist[b, t]           # [D]

v3 design (pure data parallel over batch across 8 cores):
  - Host pre-transposes hist to d-major [D, BC, T] and pre-casts to bf16:
    HBM traffic halves (105 MB/core) and the DMA needs no SWDGE cast --
    plain HWDGE with 25.6 KB/partition descriptors.  Host also ships
    tgt^T [D, BC] bf16 and q replicated to 128 columns.
  - No PE transposes at all (v2 spent ~440us of un-modeled Ldweights on
    them).  PE does only: main mm W^T x^T (512-col bf16 chunks, FWL
    hides the W reload) and per-b logits matmuls with the q128
    stationary (out = 128 replicated rows of the logit row).
  - x = hist + tgt broadcast-add on DVE (in-place, free-dim stride-0
    AP for the t-broadcast).  Pooling then uses x everywhere via the
    identity sum_t w_t x_t = pooled + (sum_t w_t) tgt  -> host subtracts
    tgt after normalizing.
  - relu+bias drains rotate GPS/DVE/ACT (constant Wb bias -> jumbo
    512-col chunks, no per-b bias needed).
  - exp on ACT reads the logits psum in batched 4-b instructions
    ([128, (2 banks, 2 b, 200)] AP) writing the 128-replicated w tile.
  - Pooling on DVE: tmp = x * w (one [128, 6400] bf16 tensor_mul per
    half-iter, 4x mode), two bf16 tree-add levels (200->50), then a
    windowed fp32 tensor_reduce -> pooled^T [d, b] in SBUF.
  - Z = sum_t w on the HOST from one shipped replica row of w
    (25.6 KB/iter); host computes pooled/Z - tgt.
"""

import sys

sys.path.insert(0, "/opt/trn_rl_repo")

import numpy as np

import concourse.bacc as bacc
import concourse.bass as bass
import concourse.mybir as mybir
import concourse.tile as tile
from concourse.bass_utils import run_bass_kernel_spmd

F32 = mybir.dt.float32
BF16 = mybir.dt.bfloat16
AF = mybir.ActivationFunctionType

NCORES = 8
B, T, D = 16384, 200, 128
BC = B // NCORES          # 2048 batch rows per core
B_IT = 64                 # batch rows per iteration
NIT = BC // B_IT          # 32
CW = B_IT * T             # 12800 columns per iteration


def build(nc, b_core=BC):
    nit = b_core // B_IT
    hist = nc.dram_tensor("histT", [D, b_core, T], BF16, kind="ExternalInput")
    tgt = nc.dram_tensor("tgtT", [D, b_core], BF16, kind="ExternalInput")
    w_in = nc.dram_tensor("W", [D, D], BF16, kind="ExternalInput")
    wb_in = nc.dram_tensor("Wb", [D], F32, kind="ExternalInput")
    q_in = nc.dram_tensor("q128", [D, D], BF16, kind="ExternalInput")
    out_p = nc.dram_tensor("out_p", [nit, D, B_IT], F32, kind="ExternalOutput")
    out_w = nc.dram_tensor("out_w", [nit, CW], BF16, kind="ExternalOutput")

    from contextlib import ExitStack
    with tile.TileContext(nc) as tc, ExitStack() as es:
        consts = es.enter_context(tc.tile_pool(name="consts", bufs=1))
        x_pool = es.enter_context(tc.tile_pool(name="x", bufs=CFG["x"]))
        hh_pool = es.enter_context(tc.tile_pool(name="hh", bufs=CFG["hh"]))
        wt_pool = es.enter_context(tc.tile_pool(name="wt", bufs=CFG["wt"]))
        tmp_pool = es.enter_context(tc.tile_pool(name="tmp", bufs=CFG["tmp"]))
        t1_pool = es.enter_context(tc.tile_pool(name="t1", bufs=2))
        t2_pool = es.enter_context(tc.tile_pool(name="t2", bufs=2))
        t3_pool = es.enter_context(tc.tile_pool(name="t3", bufs=2))
        po_pool = es.enter_context(tc.tile_pool(name="po", bufs=2))
        ps_mm = es.enter_context(tc.tile_pool(name="ps_mm", bufs=CFG["mm"], space="PSUM"))
        ps_q = es.enter_context(tc.tile_pool(name="ps_q", bufs=CFG["q"], space="PSUM"))

        # ---- constants ----
        w_sb = consts.tile([D, D], BF16)
        nc.sync.dma_start(out=w_sb, in_=w_in.ap())
        q_sb = consts.tile([D, D], BF16)
        nc.sync.dma_start(out=q_sb, in_=q_in.ap())
        wb_sb = consts.tile([D, 1], F32)
        nc.sync.dma_start(out=wb_sb, in_=wb_in.ap()[:, None])
        tgt_sb = consts.tile([D, b_core], BF16)
        nc.sync.dma_start(out=tgt_sb, in_=tgt.ap())

        drain_plan = CFG["drain"]

        for it in range(nit):
            b0 = it * B_IT

            # 1) load hist slice, d-major
            x = x_pool.tile([D, CW], BF16, tag="x")
            nc.sync.dma_start(out=x, in_=hist.ap()[:, b0:b0 + B_IT, :])

            # 2) x = hist + tgt (broadcast over t), in place
            xv = x.rearrange("d (b t) -> d b t", t=T)
            tg = tgt_sb[:, b0:b0 + B_IT]
            nc.vector.tensor_add(
                xv, xv,
                bass.AP(tensor=tg.tensor, offset=tg.offset,
                        ap=[tg.ap[0], tg.ap[1], [0, T]]),
            )

            # 3) main mm + relu drains (psum tiles span 2 banks; 2 matmuls
            #    fill bank-aligned halves, one jumbo drain empties both;
            #    GPSIMD can't touch PSUM so drains rotate ACT/DVE only)
            hh = hh_pool.tile([D, CW], BF16, tag="hh")
            for k, (c0, cn) in enumerate(
                    [(i * 1024, 1024) for i in range(CW // 1024)]
                    + ([(CW - CW % 1024, CW % 1024)] if CW % 1024 else [])):
                mm = ps_mm.tile([D, 1024], F32, tag="mm")
                for s in range(0, cn, 512):
                    nc.tensor.matmul(
                        mm[:, s:s + 512], w_sb,
                        x[:, c0 + s:c0 + s + 512],
                        start=True, stop=True, skip_group_check=True)
                eng = drain_plan[k % len(drain_plan)]
                dst = hh[:, c0:c0 + cn]
                if eng == "a":
                    nc.scalar.activation(dst, mm[:, 0:cn], AF.Relu, bias=wb_sb)
                else:
                    nc.vector.tensor_scalar(
                        dst, mm[:, 0:cn], wb_sb, 0.0,
                        mybir.AluOpType.add, mybir.AluOpType.max)

            # 4) logits (q128 stationary) + exp, 4 b's per psum tile
            hv = hh.rearrange("e (b t) -> e b t", t=T)
            wt = wt_pool.tile([D, CW], BF16, tag="wt")
            wtv = wt.rearrange("d (b t) -> d b t", t=T)
            for g in range(B_IT // 4):
                qp = ps_q.tile([D, 1024], F32, tag="qp")
                for j in range(4):
                    c0 = (j // 2) * 512 + (j % 2) * 200
                    nc.tensor.matmul(qp[:, c0:c0 + 200], q_sb,
                                     hv[:, 4 * g + j, :],
                                     start=True, stop=True,
                                     skip_group_check=True)
                qpv = bass.AP(tensor=qp.tensor, offset=qp.offset,
                              ap=[qp.ap[0], [512, 2], [200, 2], [1, 200]])
                nc.scalar.activation(
                    wtv[:, 4 * g:4 * g + 4, :].rearrange(
                        "d (gg j) t -> d gg j t", gg=2),
                    qpv, AF.Exp)

            # 5) pooling on DVE: tmp = x*w, tree-add, fp32 reduce
            pooled = po_pool.tile([D, B_IT], F32, tag="po")
            for h in range(2):
                cols = slice(h * (CW // 2), (h + 1) * (CW // 2))
                tmp = tmp_pool.tile([D, CW // 2], BF16, tag="tmp")
                nc.vector.tensor_mul(tmp, x[:, cols], wt[:, cols])
                tv = tmp.rearrange("d (b t) -> d b t", t=T)
                t1 = t1_pool.tile([D, B_IT // 2 * 100], BF16, tag="t1")
                t1v = t1.rearrange("d (b t) -> d b t", t=100)
                nc.vector.tensor_add(t1v, tv[:, :, 0:100], tv[:, :, 100:200])
                t2 = t2_pool.tile([D, B_IT // 2 * 50], BF16, tag="t2")
                t2v = t2.rearrange("d (b t) -> d b t", t=50)
                nc.vector.tensor_add(t2v, t1v[:, :, 0:50], t1v[:, :, 50:100])
                t3 = t3_pool.tile([D, B_IT // 2 * 25], BF16, tag="t3")
                t3v = t3.rearrange("d (b t) -> d b t", t=25)
                nc.vector.tensor_add(t3v, t2v[:, :, 0:25], t2v[:, :, 25:50])
                nc.vector.tensor_reduce(
                    pooled[:, h * (B_IT // 2):(h + 1) * (B_IT // 2)],
                    t3v, mybir.AxisListType.X, mybir.AluOpType.add)

            # 6) outputs: pooled^T and one replica row of w (for host Z)
            nc.sync.dma_start(out=out_p.ap()[it], in_=pooled)
            nc.sync.dma_start(out=out_w.ap()[it, :], in_=wt[0:1, :])

    return out_p


_cache = {}
LAST_RESULT = None
CFG = dict(x=2, hh=2, wt=2, tmp=2, mm=2, q=2,
           drain="adadadadadada")


def _get_program(b_core):
    key = (b_core, tuple(sorted(CFG.items())))
    if key not in _cache:
        nc = bacc.Bacc("TRN2", target_bir_lowering=False, debug=False,
                       num_devices=NCORES)
        build(nc, b_core)
        nc.compile()
        _cache[key] = nc
    return _cache[key]


def _prep_inputs(inputs):
    """Host-side layout prep: d-major bf16 hist, tgt^T, q128."""
    import ml_dtypes
    bf16 = ml_dtypes.bfloat16
    hist = np.asarray(inputs["hist_embeddings"], np.float32)
    tgt = np.asarray(inputs["target_embedding"], np.float32)
    W = np.asarray(inputs["W_kernel"], np.float32)
    Wb = np.asarray(inputs["W_bias"], np.float32)
    q = np.asarray(inputs["q_kernel"], np.float32)
    # q_bias shifts every logit equally -> softmax-invariant -> ignored.

    nc_b = hist.shape[0] // NCORES
    # [B, T, D] -> [8, D, BC, T] bf16
    histT = np.ascontiguousarray(
        hist.reshape(NCORES, nc_b, T, D).transpose(0, 3, 1, 2)
    ).astype(bf16)
    tgtT = np.ascontiguousarray(
        tgt.reshape(NCORES, nc_b, D).transpose(0, 2, 1)).astype(bf16)
    W_bf = W.astype(bf16)
    q128 = np.ascontiguousarray(np.repeat(q.astype(bf16), D, axis=1))
    return histT, tgtT, W_bf, Wb, q128, tgt


def decode_out(res_p, res_w, tgt_core, b_core=BC):
    """out_p [nit, D, B_IT] f32, out_w [nit, CW] bf16 -> out [b_core, D]."""
    nit = b_core // B_IT
    pooled = np.asarray(res_p, np.float32).transpose(0, 2, 1).reshape(b_core, D)
    w = np.asarray(res_w).astype(np.float32).reshape(nit, B_IT, T)
    Z = w.sum(axis=2).reshape(b_core)
    return pooled / Z[:, None] - tgt_core


def kernel(**inputs):
    histT, tgtT, W_bf, Wb, q128, tgt = _prep_inputs(inputs)
    nc = _get_program(BC)
    in_maps = []
    for c in range(NCORES):
        in_maps.append({
            "histT": histT[c], "tgtT": tgtT[c],
            "W": W_bf, "Wb": Wb, "q128": q128,
        })
    res = run_bass_kernel_spmd(nc, in_maps, core_ids=list(range(NCORES)))
    global LAST_RESULT
    LAST_RESULT = res
    outs = []
    for c in range(NCORES):
        outs.append(decode_out(res.results[c]["out_p"],
                               res.results[c]["out_w"],
                               tgt[c * BC:(c + 1) * BC]))
    return np.concatenate(outs, axis=0).astype(np.float32)


def timed_run(inputs, iters=5, bcs=BC):
    """Device-resident repeated execution; returns (best_seconds, outputs)."""
    import time
    import jax
    from jax.sharding import Mesh, PartitionSpec
    from jax.experimental.shard_map import shard_map
    import concourse.mybir as mybir_
    from concourse.bass2jax import (install_neuronx_cc_hook, _bass_exec_p,
                                    partition_id_tensor)

    histT, tgtT, W_bf, Wb, q128, tgt = _prep_inputs(inputs)
    nc = _get_program(bcs)
    install_neuronx_cc_hook()

    pid_name = nc.partition_id_tensor.name if nc.partition_id_tensor else None
    in_names, out_names, out_avals, zero_outs = [], [], [], []
    for alloc in nc.m.functions[0].allocations:
        if not isinstance(alloc, mybir_.MemoryLocationSet):
            continue
        name = alloc.memorylocations[0].name
        if alloc.kind == "ExternalInput":
            if name != pid_name:
                in_names.append(name)
        elif alloc.kind == "ExternalOutput":
            shape = tuple(alloc.tensor_shape)
            dtype = mybir_.dt.np(alloc.dtype)
            out_names.append(name)
            out_avals.append(jax.core.ShapedArray(shape, dtype))
            zero_outs.append(np.zeros(shape, dtype))
    all_names = in_names + out_names
    if pid_name is not None:
        all_names = all_names + [pid_name]

    import os
    chain = int(os.environ.get("KERNEL_CHAIN", "1"))

    aliases = tuple((oi, len(in_names) + oi) for oi in range(len(out_names)))

    def _body(*args):
        nin_ = len(in_names)
        ins_ = list(args[:nin_])
        outs = list(args[nin_:])
        for _ in range(chain):
            operands = ins_ + outs
            if pid_name is not None:
                operands = operands + [partition_id_tensor()]
            outs = list(_bass_exec_p.bind(
                *operands, out_avals=tuple(out_avals),
                in_names=tuple(all_names), out_names=tuple(out_names),
                lowering_input_output_aliases=aliases,
                sim_require_finite=True, sim_require_nnan=True, nc=nc))
        return tuple(outs)

    devices = jax.devices()[:NCORES]
    mesh = Mesh(np.array(devices), ("core",))
    nin = len(in_names) + len(out_names)
    fn = jax.jit(shard_map(_body, mesh=mesh,
                           in_specs=(PartitionSpec("core"),) * nin,
                           out_specs=(PartitionSpec("core"),) * len(out_names),
                           check_rep=False),
                 donate_argnums=tuple(range(len(in_names), nin)))
    full = {"histT": histT.reshape(-1, *histT.shape[2:]),
            "tgtT": tgtT.reshape(-1, *tgtT.shape[2:]),
            "W": np.concatenate([W_bf] * NCORES, 0),
            "Wb": np.concatenate([Wb] * NCORES, 0),
            "q128": np.concatenate([q128] * NCORES, 0)}
    args = [full[n] for n in in_names] + [
        np.concatenate([z] * NCORES, 0) for z in zero_outs]
    sh = jax.sharding.NamedSharding(mesh, PartitionSpec("core"))
    dargs = [jax.device_put(a, sh) for a in args]
    r = fn(*dargs)
    jax.block_until_ready(r)
    pipeline = int(os.environ.get("KERNEL_PIPE", "1"))
    nin_ = len(in_names)
    best = float("inf")
    for _ in range(iters):
        t0 = time.perf_counter()
        for _k in range(pipeline):
            r = fn(*dargs[:nin_], *r)
        jax.block_until_ready(r)
        best = min(best, time.perf_counter() - t0)
    outs = [np.asarray(x) for x in r]
    per_p = np.split(outs[out_names.index("out_p")], NCORES, axis=0)
    per_w = np.split(outs[out_names.index("out_w")], NCORES, axis=0)
    full_out = []
    for c in range(NCORES):
        full_out.append(decode_out(per_p[c], per_w[c],
                                   tgt[c * bcs:(c + 1) * bcs], bcs))
    return best, np.concatenate(full_out, 0).astype(np.float32)


if __name__ == "__main__":
    rng = np.random.default_rng(0)
    ins = {
        "target_embedding": rng.standard_normal((B, D), dtype=np.float32),
        "hist_embeddings": rng.standard_normal((B, T, D), dtype=np.float32),
        "W_kernel": (rng.standard_normal((D, D), dtype=np.float32) / np.sqrt(D)),
        "W_bias": np.zeros(D, np.float32),
        "q_kernel": (rng.standard_normal((D, 1), dtype=np.float32) / np.sqrt(D)),
        "q_bias": np.zeros(1, np.float32),
    }
    out = kernel(**ins)
    print("out", out.shape, out.dtype)


# revision 11
# speedup vs baseline: 1.1876x; 1.1876x over previous
"""AttentionPooling Trainium2 kernel, v4 ("D2": dual-layout bf16, PE pool).

Math (per batch row b):
    x   = target[b] + hist[b]              # [T, D]
    h   = relu(x @ W + Wb)                 # [T, D]
    lg  = h @ q  (+ q_bias, softmax-invariant -> ignored)
    s   = softmax(lg) over T
    out = sum_t s_t * hist[b, t]           # [D]

v4 design (pure data parallel over batch across 8 cores):
  - Host pre-casts hist to bf16 and ships it in BOTH layouts:
    d-major histT [D, BC, T] (feeds the W matmul; 25.6 KB/partition
    descriptors) and t2-parity histP [T2, BC, 2, D] (feeds the PE
    pooling matmuls; 32 KB descriptors).  2x bf16 = same HBM bytes as
    the old fp32 single load, but no PE transposes (v2 spent ~440us of
    sim-invisible Ldweights on them) and no SWDGE cast.
  - x = hist + tgt on the d-major copy in place (DVE/GPS split, packed
    tgx APs for the DVE 2x mode).  The t-major copy stays pristine, so
    pooling uses exact hist (no cancellation-amplified error).
  - PE: main mm (512-col bf16 chunks, FWL-hidden W reload), q32 logits
    with tile_position 4-b packing, w transposes (par-strided slices),
    pooling matmuls (w32 stationary, psum par-accumulated).
  - relu+bias drains rotate ACT/DVE in [128,1024] jumbo chunks.
  - exp on ACT per (gp, gg) [128, 200] with accum_out -> Z on device
    for free.  Host only normalizes: out = pooled / Z.
"""

import sys

sys.path.insert(0, "/opt/trn_rl_repo")

import numpy as np

import concourse.bacc as bacc
import concourse.bass as bass
import concourse.mybir as mybir
import concourse.tile as tile
from concourse.bass_utils import run_bass_kernel_spmd

F32 = mybir.dt.float32
BF16 = mybir.dt.bfloat16
AF = mybir.ActivationFunctionType

NCORES = 8
B, T, D = 16384, 200, 128
T2 = T // 2               # 100
BC = B // NCORES          # 2048 batch rows per core
B_IT = 64                 # batch rows per iteration
NIT = BC // B_IT          # 32
CW = B_IT * T             # 12800 columns per iteration
NGP = B_IT // 8           # 8 logit groups of 8 b's


def build(nc, b_core=BC):
    nit = b_core // B_IT
    histT = nc.dram_tensor("histT", [D, b_core, T], BF16, kind="ExternalInput")
    histP = nc.dram_tensor("histP", [T2, b_core, 2, D], BF16, kind="ExternalInput")
    tgt = nc.dram_tensor("tgtT", [D, b_core], BF16, kind="ExternalInput")
    w_in = nc.dram_tensor("W", [D, D], BF16, kind="ExternalInput")
    wb_in = nc.dram_tensor("Wb", [D], F32, kind="ExternalInput")
    q_in = nc.dram_tensor("q32", [D, 32], BF16, kind="ExternalInput")
    out_p = nc.dram_tensor("out_p", [nit, 4, NGP * 2 * D], BF16, kind="ExternalOutput")
    out_z = nc.dram_tensor("out_z", [nit, D, NGP * 2], F32, kind="ExternalOutput")

    from contextlib import ExitStack
    with tile.TileContext(nc) as tc, ExitStack() as es:
        consts = es.enter_context(tc.tile_pool(name="consts", bufs=1))
        x_pool = es.enter_context(tc.tile_pool(name="x", bufs=2))
        nt_pool = es.enter_context(tc.tile_pool(name="nt", bufs=2))
        hh_pool = es.enter_context(tc.tile_pool(name="hh", bufs=2))
        wt_pool = es.enter_context(tc.tile_pool(name="wt", bufs=3))
        ws_pool = es.enter_context(tc.tile_pool(name="ws", bufs=3))
        tgx_pool = es.enter_context(tc.tile_pool(name="tgx", bufs=2))
        out_pool = es.enter_context(tc.tile_pool(name="out", bufs=2))
        z_pool = es.enter_context(tc.tile_pool(name="z", bufs=2))
        ps_mm = es.enter_context(tc.tile_pool(name="ps_mm", bufs=2, space="PSUM"))
        ps_q = es.enter_context(tc.tile_pool(name="ps_q", bufs=2, space="PSUM"))
        ps_wt = es.enter_context(tc.tile_pool(name="ps_wt", bufs=1, space="PSUM"))
        ps_pp = es.enter_context(tc.tile_pool(name="ps_pp", bufs=1, space="PSUM"))

        # ---- constants ----
        w_sb = consts.tile([D, D], BF16)
        nc.sync.dma_start(out=w_sb, in_=w_in.ap())
        q_sb = consts.tile([D, 32], BF16)
        nc.sync.dma_start(out=q_sb, in_=q_in.ap())
        wb_sb = consts.tile([D, 1], F32)
        nc.sync.dma_start(out=wb_sb, in_=wb_in.ap()[:, None])
        tgt_sb = consts.tile([D, b_core], BF16)
        nc.sync.dma_start(out=tgt_sb, in_=tgt.ap())
        from concourse import masks
        ident = consts.tile([128, 128], BF16)
        masks.make_identity(nc, ident[:, :])

        drain_plan = CFG["drain"]
        add_gps = CFG["add_gps"]          # fraction (in 1/8ths) of add on GPS

        for it in range(nit):
            b0 = it * B_IT

            # ---- loads ----
            x = x_pool.tile([D, CW], BF16, tag="x")
            nc.sync.dma_start(out=x, in_=histT.ap()[:, b0:b0 + B_IT, :])
            nt = nt_pool.tile([T2, B_IT * 2 * D], BF16, tag="nt")
            nc.sync.dma_start(
                out=nt, in_=histP.ap()[:, b0:b0 + B_IT, :, :].rearrange(
                    "t b p e -> t (b p e)"))
            ntv = nt.rearrange("t (b p e) -> t b p e", p=2, e=D)

            # ---- x = hist + tgt (in place on the d-major copy) ----
            # tgx: tgt slice expanded 8x so the add's in1 has a packed
            # last dim (DVE 2x mode needs stride-1)
            tgx = tgx_pool.tile([D, B_IT * 8], BF16, tag="tgx")
            sl = tgt_sb[:, b0:b0 + B_IT]
            nc.vector.tensor_copy(
                out=tgx,
                in_=bass.AP(tensor=sl.tensor, offset=sl.offset,
                            ap=[sl.ap[0], sl.ap[1], [0, 8]]))
            tgxv = tgx.rearrange("d (b r) -> d b r", r=8)
            xv = x.rearrange("d (b o i) -> d b o i", b=B_IT, i=8)
            nsplit = (B_IT * add_gps) // 8
            for eng, lo, hi in ((nc.gpsimd, 0, nsplit),
                                (nc.vector, nsplit, B_IT)):
                if lo == hi:
                    continue
                tg4 = tgxv[:, lo:hi, :]
                eng.tensor_add(
                    xv[:, lo:hi],
                    xv[:, lo:hi],
                    bass.AP(tensor=tg4.tensor, offset=tg4.offset,
                            ap=[tg4.ap[0], tg4.ap[1], [0, T // 8], tg4.ap[2]]))

            # ---- main mm + relu drains (ACT/DVE rotation) ----
            hh = hh_pool.tile([D, CW], BF16, tag="hh")
            for k, (c0, cn) in enumerate(
                    [(i * 1024, 1024) for i in range(CW // 1024)]
                    + ([(CW - CW % 1024, CW % 1024)] if CW % 1024 else [])):
                mm = ps_mm.tile([D, 1024], F32, tag="mm")
                for s in range(0, cn, 512):
                    nc.tensor.matmul(
                        mm[:, s:s + 512], w_sb,
                        x[:, c0 + s:c0 + s + 512],
                        start=True, stop=True, skip_group_check=True)
                eng = drain_plan[k % len(drain_plan)]
                dst = hh[:, c0:c0 + cn]
                if eng == "a":
                    nc.scalar.activation(dst, mm[:, 0:cn], AF.Relu, bias=wb_sb)
                else:
                    nc.vector.tensor_scalar(
                        dst, mm[:, 0:cn], wb_sb, 0.0,
                        mybir.AluOpType.add, mybir.AluOpType.max)

            # ---- logits (q32, 4-b tile_position packing) + exp + Z ----
            hv = hh.rearrange("e (b t) -> e b t", t=T)
            ztile = z_pool.tile([D, NGP * 2], F32, tag="z")
            wtiles = {}
            for gp in range(NGP):
                qp = ps_q.tile([D, 2 * T], F32, tag="qp")
                for gg in range(2):
                    g = 2 * gp + gg
                    for j in range(4):
                        nc.tensor.matmul(
                            qp[32 * j:32 * j + 32, gg * T:(gg + 1) * T],
                            q_sb, hv[:, 4 * g + j, :],
                            start=True, stop=True,
                            skip_group_check=True,
                            tile_position=(0, 32 * j))
                wtile = wt_pool.tile([D, 2 * T], BF16, tag="wt")
                for gg in range(2):
                    nc.scalar.activation(
                        wtile[:, gg * T:(gg + 1) * T],
                        qp[:, gg * T:(gg + 1) * T], AF.Exp,
                        accum_out=ztile[:, 2 * gp + gg:2 * gp + gg + 1])
                wtiles[gp] = wtile

            # ---- w transposes + pooling matmuls ----
            outt = out_pool.tile([D, NGP * 2 * D], BF16, tag="outt")
            for gp in range(NGP):
                wtile = wtiles[gp]
                # wT [t2, (gg,par) x 128 (j,rep) cols]; stationary slices are
                # par-strided (t natural order: t = 2*t2 + par)
                wt_ps = ps_wt.tile([T2, 512], BF16, tag="wtp")
                for gg in range(2):
                    for par in range(2):
                        w0 = wtile[:, gg * T + par:gg * T + par + 1]
                        wsl = bass.AP(tensor=w0.tensor, offset=w0.offset,
                                      ap=[w0.ap[0], [2, T2]])
                        nc.tensor.transpose(
                            wt_ps[:, (2 * gg + par) * 128:
                                  (2 * gg + par) * 128 + 128],
                            wsl, ident)
                wt_sb = ws_pool.tile([T2, 512], BF16, tag="ws")
                nc.vector.tensor_copy(out=wt_sb, in_=wt_ps)
                pp = ps_pp.tile([D, 2 * D], F32, tag="pp")
                for gg in range(2):
                    g = 2 * gp + gg
                    for j in range(4):
                        bb = 4 * g + j

                        def st32(par):
                            return wt_sb[:, (2 * gg + par) * 128 + 32 * j:
                                         (2 * gg + par) * 128 + 32 * j + 32]

                        nc.tensor.matmul(
                            pp[32 * j:32 * j + 32, D * gg:D * (gg + 1)],
                            st32(0), ntv[:, bb, 0, :],
                            start=True, stop=False,
                            skip_group_check=True,
                            tile_position=(0, 32 * j))
                        nc.tensor.matmul(
                            pp[32 * j:32 * j + 32, D * gg:D * (gg + 1)],
                            st32(1), ntv[:, bb, 1, :],
                            start=False, stop=True,
                            skip_group_check=True,
                            tile_position=(0, 32 * j))
                dst = outt[:, 2 * D * gp:2 * D * (gp + 1)]
                if gp % 2 == 0:
                    nc.vector.tensor_copy(out=dst, in_=pp)
                else:
                    nc.scalar.activation(dst, pp, AF.Copy)

            # ---- outputs ----
            for j in range(4):
                nc.sync.dma_start(out=out_p.ap()[it, j, :],
                                  in_=outt[32 * j:32 * j + 1, :])
            nc.sync.dma_start(out=out_z.ap()[it], in_=ztile)

    return out_p


_cache = {}
LAST_RESULT = None
CFG = dict(drain="adadadadadada", add_gps=3)


def _get_program(b_core):
    key = (b_core, tuple(sorted(CFG.items())))
    if key not in _cache:
        nc = bacc.Bacc("TRN2", target_bir_lowering=False, debug=False,
                       num_devices=NCORES)
        build(nc, b_core)
        nc.compile()
        _cache[key] = nc
    return _cache[key]


def _prep_inputs(inputs):
    """Host-side layout prep: bf16 hist in d-major AND t2-parity layouts."""
    import ml_dtypes
    bf16 = ml_dtypes.bfloat16
    hist = np.asarray(inputs["hist_embeddings"], np.float32)
    tgt = np.asarray(inputs["target_embedding"], np.float32)
    W = np.asarray(inputs["W_kernel"], np.float32)
    Wb = np.asarray(inputs["W_bias"], np.float32)
    q = np.asarray(inputs["q_kernel"], np.float32)
    # q_bias shifts every logit equally -> softmax-invariant -> ignored.

    nc_b = hist.shape[0] // NCORES
    hist_bf = hist.astype(bf16)
    # [B, T, D] -> [8, D, BC, T]
    histT = np.ascontiguousarray(
        hist_bf.reshape(NCORES, nc_b, T, D).transpose(0, 3, 1, 2))
    # [B, T, D] -> [B, T2, 2, D] -> [8, T2, BC, 2, D]
    histP = np.ascontiguousarray(
        hist_bf.reshape(NCORES, nc_b, T2, 2, D).transpose(0, 2, 1, 3, 4))
    tgtT = np.ascontiguousarray(
        tgt.reshape(NCORES, nc_b, D).transpose(0, 2, 1)).astype(bf16)
    W_bf = W.astype(bf16)
    q32 = np.ascontiguousarray(np.repeat(q.astype(bf16), 32, axis=1))
    return histT, histP, tgtT, W_bf, Wb, q32


def decode_out(res_p, res_z, b_core=BC):
    """out_p [nit,4,NGP*2*D] bf16 (row j), out_z [nit,D,NGP*2] -> [b_core, D]."""
    nit = b_core // B_IT
    p = np.asarray(res_p).astype(np.float32).reshape(nit, 4, NGP, 2, D)
    # b = 64*it + 4*(2*gp+gg) + j  -> order (it, gp, gg, j)
    p = p.transpose(0, 2, 3, 1, 4).reshape(b_core, D)
    z = np.asarray(res_z, np.float32)                   # [nit, D, NGP*2]
    idx = np.arange(4)
    zj = z.reshape(nit, 4, 32, NGP, 2)[:, idx, 0][:, :, :, :]  # [nit,4,NGP,2] rows 32j
    Z = zj.transpose(0, 2, 3, 1).reshape(b_core)
    return p / Z[:, None]


def kernel(**inputs):
    histT, histP, tgtT, W_bf, Wb, q32 = _prep_inputs(inputs)
    nc = _get_program(BC)
    in_maps = []
    for c in range(NCORES):
        in_maps.append({
            "histT": histT[c], "histP": histP[c], "tgtT": tgtT[c],
            "W": W_bf, "Wb": Wb, "q32": q32,
        })
    res = run_bass_kernel_spmd(nc, in_maps, core_ids=list(range(NCORES)))
    global LAST_RESULT
    LAST_RESULT = res
    outs = []
    for c in range(NCORES):
        outs.append(decode_out(res.results[c]["out_p"],
                               res.results[c]["out_z"]))
    return np.concatenate(outs, axis=0).astype(np.float32)


def timed_run(inputs, iters=5, bcs=BC):
    """Device-resident repeated execution; returns (best_seconds, outputs)."""
    import time
    import jax
    from jax.sharding import Mesh, PartitionSpec
    from jax.experimental.shard_map import shard_map
    import concourse.mybir as mybir_
    from concourse.bass2jax import (install_neuronx_cc_hook, _bass_exec_p,
                                    partition_id_tensor)

    histT, histP, tgtT, W_bf, Wb, q32 = _prep_inputs(inputs)
    nc = _get_program(bcs)
    install_neuronx_cc_hook()

    pid_name = nc.partition_id_tensor.name if nc.partition_id_tensor else None
    in_names, out_names, out_avals, zero_outs = [], [], [], []
    for alloc in nc.m.functions[0].allocations:
        if not isinstance(alloc, mybir_.MemoryLocationSet):
            continue
        name = alloc.memorylocations[0].name
        if alloc.kind == "ExternalInput":
            if name != pid_name:
                in_names.append(name)
        elif alloc.kind == "ExternalOutput":
            shape = tuple(alloc.tensor_shape)
            dtype = mybir_.dt.np(alloc.dtype)
            out_names.append(name)
            out_avals.append(jax.core.ShapedArray(shape, dtype))
            zero_outs.append(np.zeros(shape, dtype))
    all_names = in_names + out_names
    if pid_name is not None:
        all_names = all_names + [pid_name]

    import os
    chain = int(os.environ.get("KERNEL_CHAIN", "1"))

    aliases = tuple((oi, len(in_names) + oi) for oi in range(len(out_names)))

    def _body(*args):
        nin_ = len(in_names)
        ins_ = list(args[:nin_])
        outs = list(args[nin_:])
        for _ in range(chain):
            operands = ins_ + outs
            if pid_name is not None:
                operands = operands + [partition_id_tensor()]
            outs = list(_bass_exec_p.bind(
                *operands, out_avals=tuple(out_avals),
                in_names=tuple(all_names), out_names=tuple(out_names),
                lowering_input_output_aliases=aliases,
                sim_require_finite=True, sim_require_nnan=True, nc=nc))
        return tuple(outs)

    devices = jax.devices()[:NCORES]
    mesh = Mesh(np.array(devices), ("core",))
    nin = len(in_names) + len(out_names)
    fn = jax.jit(shard_map(_body, mesh=mesh,
                           in_specs=(PartitionSpec("core"),) * nin,
                           out_specs=(PartitionSpec("core"),) * len(out_names),
                           check_rep=False),
                 donate_argnums=tuple(range(len(in_names), nin)))
    full = {"histT": histT.reshape(-1, *histT.shape[2:]),
            "histP": histP.reshape(-1, *histP.shape[2:]),
            "tgtT": tgtT.reshape(-1, *tgtT.shape[2:]),
            "W": np.concatenate([W_bf] * NCORES, 0),
            "Wb": np.concatenate([Wb] * NCORES, 0),
            "q32": np.concatenate([q32] * NCORES, 0)}
    args = [full[n] for n in in_names] + [
        np.concatenate([z] * NCORES, 0) for z in zero_outs]
    sh = jax.sharding.NamedSharding(mesh, PartitionSpec("core"))
    dargs = [jax.device_put(a, sh) for a in args]
    r = fn(*dargs)
    jax.block_until_ready(r)
    pipeline = int(os.environ.get("KERNEL_PIPE", "1"))
    nin_ = len(in_names)
    best = float("inf")
    for _ in range(iters):
        t0 = time.perf_counter()
        for _k in range(pipeline):
            r = fn(*dargs[:nin_], *r)
        jax.block_until_ready(r)
        best = min(best, time.perf_counter() - t0)
    outs = [np.asarray(x) for x in r]
    per_p = np.split(outs[out_names.index("out_p")], NCORES, axis=0)
    per_z = np.split(outs[out_names.index("out_z")], NCORES, axis=0)
    full_out = []
    for c in range(NCORES):
        full_out.append(decode_out(per_p[c], per_z[c], bcs))
    return best, np.concatenate(full_out, 0).astype(np.float32)


if __name__ == "__main__":
    rng = np.random.default_rng(0)
    ins = {
        "target_embedding": rng.standard_normal((B, D), dtype=np.float32),
        "hist_embeddings": rng.standard_normal((B, T, D), dtype=np.float32),
        "W_kernel": (rng.standard_normal((D, D), dtype=np.float32) / np.sqrt(D)),
        "W_bias": np.zeros(D, np.float32),
        "q_kernel": (rng.standard_normal((D, 1), dtype=np.float32) / np.sqrt(D)),
        "q_bias": np.zeros(1, np.float32),
    }
    out = kernel(**ins)
    print("out", out.shape, out.dtype)


# revision 12
# speedup vs baseline: 4.0556x; 3.4150x over previous
"""AttentionPooling Trainium2 kernel, v2.

Math (per batch row b):
    x   = target[b] + hist[b]              # [T, D]
    h   = relu(x @ W + Wb)                 # [T, D]
    lg  = h @ q  (+ q_bias, softmax-invariant -> ignored)
    s   = softmax(lg) over T
    out = sum_t s_t * hist[b, t]           # [D]

v2 design (pure data parallel over batch across 8 cores):
  - T2 "parity" layout: t = 2*t2 + par; hist loaded HBM->SBUF with
    fp32->bf16 cast (SWDGE) into nt [t2=100 part, (b, par, d)] — the
    (par, d) pairs are 1024B-contiguous in HBM, halving descriptor count
    vs the 512B [t,(b,d)] layout (measured ~571us vs ~717us per exec;
    2KB descriptors with cast collapse, so T2 is the sweet spot).
  - PE transposes nt -> xT [d, (b, par, t2)] fused with broadcast-add of
    targetT on DVE (psum drain).
  - Main matmul: H^T = W^T @ xT (bf16, W stationary). The relu+bias psum
    drains rotate ACT/ACT/DVE ("aad") — a lone ACT (~830ns per 512-col
    activation on HW) would be the bottleneck engine.
  - Logits via q replicated 32x (stationary q32): per 4-b group, 200-col
    matmuls land logits [32-replicated rows, t]; one exp per group-pair
    on ACT.
  - w transposed back (PE) to [t2, 32-replica cols] for use as the
    pooling stationary (psum->sbuf staging on DVE).
  - Pooling matmul per b accumulates par0+par1 into psum rows 32*(b%4);
    per-pair wsum via one ones-stationary matmul over 32-strided replica
    columns.  Drains alternate DVE/ACT; bf16 out_dev halves the output
    traffic (PJRT output handling is a large fixed cost per exec).
  - Final normalize (divide by wsum) + layout de-permute on host.
"""

import sys

sys.path.insert(0, "/opt/trn_rl_repo")

import numpy as np

import concourse.bacc as bacc
import concourse.bass as bass
import concourse.mybir as mybir
import concourse.tile as tile
from concourse import masks
from concourse.bass_utils import run_bass_kernel_spmd

F32 = mybir.dt.float32
BF16 = mybir.dt.bfloat16
AF = mybir.ActivationFunctionType

NCORES = 8
B, T, D = 16384, 200, 128
BC = B // NCORES          # 2048 batch rows per core
T2 = T // 2               # 100 t2 partitions, 2 parities
E1 = D + 1                # d cols + ones col
B_IT = 64                 # batch rows per outer iteration
NSUB = B_IT // 4          # 16 sub-blocks of 4 b's (transposes)
NGRP = B_IT // 4          # 16 groups of 4 b's (pool)
GW = 2 * D + 16              # per-g2 out cols: 2x128 pooled + 16 wsums
OUTW = (NGRP // 2) * GW      # 2176 out cols per iter


def build(nc, b_core=BC):
    nit = b_core // B_IT
    hist = nc.dram_tensor("hist", [b_core, T, D], F32, kind="ExternalInput")
    tgt = nc.dram_tensor("target", [b_core, D], F32, kind="ExternalInput")
    w_in = nc.dram_tensor("W", [D, D], F32, kind="ExternalInput")
    wb_in = nc.dram_tensor("Wb", [D], F32, kind="ExternalInput")
    q_in = nc.dram_tensor("q", [D, 1], F32, kind="ExternalInput")
    out_dev = nc.dram_tensor("out_dev", [nit, 4, OUTW], BF16, kind="ExternalOutput")

    from contextlib import ExitStack
    with tile.TileContext(nc) as tc, ExitStack() as es:
        consts = es.enter_context(tc.tile_pool(name="consts", bufs=1))
        nt_pool = es.enter_context(tc.tile_pool(name="nt", bufs=1))
        ht_pool = es.enter_context(tc.tile_pool(name="ht", bufs=CFG["ht"]))
        h_pool = es.enter_context(tc.tile_pool(name="h", bufs=CFG["hh"]))
        w_pool = es.enter_context(tc.tile_pool(name="w", bufs=CFG["w"]))
        out_pool = es.enter_context(tc.tile_pool(name="out", bufs=CFG["outt"]))
        ps_tp = es.enter_context(tc.tile_pool(name="ps_tp", bufs=CFG["tp"], space="PSUM"))
        ps_mm = es.enter_context(tc.tile_pool(name="ps_mm", bufs=CFG["mm"], space="PSUM"))
        ps_q = es.enter_context(tc.tile_pool(name="ps_q", bufs=CFG["q"], space="PSUM"))
        ps_pool = es.enter_context(tc.tile_pool(name="ps_pool", bufs=CFG["pool"], space="PSUM"))

        # ---- constants ----
        ident = consts.tile([128, 128], BF16)
        masks.make_identity(nc, ident[:, :])

        w_f32 = consts.tile([D, D], F32)
        nc.sync.dma_start(out=w_f32, in_=w_in.ap())
        w_bf = consts.tile([D, D], BF16)
        nc.vector.tensor_copy(out=w_bf, in_=w_f32)

        wbias = consts.tile([D, 1], F32)
        nc.sync.dma_start(out=wbias, in_=wb_in.ap()[:, None])

        q_f32 = consts.tile([D, 1], F32)
        nc.sync.dma_start(out=q_f32, in_=q_in.ap())
        q_bf = consts.tile([D, 1], BF16)
        nc.vector.tensor_copy(out=q_bf, in_=q_f32)
        q32 = consts.tile([D, 32], BF16)
        nc.vector.tensor_copy(
            out=q32,
            in_=bass.AP(tensor=q_bf.tensor, offset=q_bf.offset,
                        ap=[q_bf.ap[0], [0, 32]]),
        )

        # targetT [d, b_core] bf16
        tgtT = consts.tile([D, b_core], BF16)
        for k in range((b_core + 127) // 128):
            bn = min(128, b_core - k * 128)
            t_f32 = w_pool.tile([128, D], F32, tag="tsetup", bufs=2)
            nc.sync.dma_start(out=t_f32[0:bn], in_=tgt.ap()[k * 128:k * 128 + bn, :])
            t_bf = w_pool.tile([128, D], BF16, tag="tsetup_bf", bufs=2)
            nc.vector.tensor_copy(out=t_bf[0:bn], in_=t_f32[0:bn])
            tp = ps_tp.tile([128, 4 * T], BF16, tag="tp")
            nc.tensor.transpose(tp[:, 0:bn], t_bf[0:bn], ident[0:bn, 0:bn])
            nc.vector.tensor_copy(out=tgtT[:, k * 128:k * 128 + bn], in_=tp[:, 0:bn])

        # ones block for the per-g2 wsum matmul (all 128 out rows written
        # so the psum drain copy reads fully-initialized data)
        ones128 = consts.tile([T2, 128], BF16)
        nc.vector.memset(ones128, 1.0)

        # persistent nt buffers: [t2, (b, par, d)] — (par, d) contiguous in
        # HBM so the cast DMA gets 1024B descriptors
        nt_bufs = []
        for nb in range(2):
            ntb = nt_pool.tile([T2, B_IT * 2 * D], BF16, tag=f"nt{nb}")
            nt_bufs.append(ntb)

        # ---- main loop ----
        for it in range(nit):
            b0 = it * B_IT
            nt = nt_bufs[it % 2]
            ntv = nt.rearrange("t (b p e) -> t b p e", p=2, e=D)
            bs = B_IT // 2
            for s in range(2):
                if CFG.get("tiny_dma"):
                    nc.gpsimd.dma_start(
                        out=ntv[0:1, s * bs:s * bs + 1, 0, 0:D],
                        in_=hist.ap()[b0:b0 + 1, 0:1, :]
                        .rearrange("b t d -> t b d"))
                    continue
                nc.gpsimd.dma_start(
                    out=ntv[:, s * bs:(s + 1) * bs, :, :].rearrange(
                        "t b p e -> t b (p e)"),
                    in_=hist.ap()[b0 + s * bs:b0 + (s + 1) * bs, :, :]
                    .rearrange("b (t x) d -> t b (x d)", x=2),
                )

            # targetT expanded 8x along t for an aligned broadcast-add AP
            tgx = w_pool.tile([128, B_IT * 8], BF16, tag="tgx")
            sl = tgtT[:, b0:b0 + B_IT]
            nc.vector.tensor_copy(
                out=tgx,
                in_=bass.AP(tensor=sl.tensor, offset=sl.offset,
                            ap=[sl.ap[0], sl.ap[1], [0, 8]]),
            )
            tgxv = tgx.rearrange("d (b r) -> d b r", r=8)

            # histT + targetT broadcast -> xT [d, (b, par, t2)]
            ht = ht_pool.tile([128, B_IT * T], BF16, tag="ht")
            htv = ht.rearrange("d (b t) -> d b t", t=T)
            for m in range(NSUB) if "tp" not in SKIP else []:
                tp = ps_tp.tile([128, 4 * T], BF16, tag="tp")
                tpv = tp.rearrange("d (b t) -> d b t", t=T)
                for bl in range(4):
                    bb = 4 * m + bl
                    for par in range(2):
                        nc.tensor.transpose(
                            tpv[:, bl, par * T2:(par + 1) * T2],
                            ntv[:, bb, par, 0:D],
                            ident[0:T2, 0:T2])
                hts = htv[:, 4 * m:4 * m + 4, :]
                tg4 = tgxv[:, 4 * m:4 * m + 4, :]
                nc.vector.tensor_add(
                    hts.rearrange("d b (to ti) -> d b to ti", ti=8),
                    tp.rearrange("d (b to ti) -> d b to ti", b=4, ti=8),
                    bass.AP(tensor=tg4.tensor, offset=tg4.offset,
                            ap=[tg4.ap[0], tg4.ap[1], [0, T // 8], tg4.ap[2]]),
                )

            # H^T = relu(W^T xT + bias)  [e, (b, par, t2)]
            # psum drains rotate across ACT / DVE / GPSIMD: the ACT engine
            # alone (~830ns per 512-col activation) would be the bottleneck
            hh = h_pool.tile([128, B_IT * T], BF16, tag="hh")
            nmm = (B_IT * T) // 512
            drain_plan = CFG.get("drain", "aad")
            for k in range(nmm) if "mm" not in SKIP else []:
                mm = ps_mm.tile([128, 512], F32, tag="mm")
                nc.tensor.matmul(mm, w_bf, ht[:, 512 * k:512 * (k + 1)],
                                 start=True, stop=True)
                eng = drain_plan[k % len(drain_plan)]
                dst = hh[:, 512 * k:512 * (k + 1)]
                if eng == "a":
                    nc.scalar.activation(dst, mm, AF.Relu, bias=wbias)
                elif eng == "d":
                    nc.vector.tensor_scalar(
                        dst, mm, wbias, 0.0,
                        mybir.AluOpType.add, mybir.AluOpType.max)
                else:
                    nc.gpsimd.tensor_scalar(
                        dst, mm, wbias, 0.0,
                        mybir.AluOpType.add, mybir.AluOpType.max)

            # logits via q32 (wide moving, 32-replicated rows), exp on ACT;
            # wtile [32-repl rows, (g-pair, t)] in (par, t2) order
            hv = hh.rearrange("e (b t) -> e b t", t=T)
            wtiles = {}
            if "q" not in SKIP:
                for gp in range(NGRP // 2):
                    qp = ps_q.tile([128, 2 * T], F32, tag="lg")
                    for gg in range(2):
                        g = 2 * gp + gg
                        for j in range(4):
                            nc.tensor.matmul(
                                qp[32 * j:32 * j + 32,
                                   gg * T:(gg + 1) * T],
                                q32, hv[:, 4 * g + j, :],
                                start=True, stop=True,
                                skip_group_check=True,
                                tile_position=(0, 32 * j))
                    wtile = w_pool.tile([128, 2 * T], BF16, tag="wtile")
                    nc.scalar.activation(wtile, qp, AF.Exp)
                    wtiles[gp] = wtile

            # pooling: per g2 transpose w to [t2, cols] then accumulate pars
            outt = out_pool.tile([128, OUTW], BF16, tag="outt")
            for g2 in range(NGRP // 2) if "pool" not in SKIP else []:
                wtile = wtiles[g2]
                wt_ps = ps_tp.tile([T2, 512], BF16, tag="tp")
                for gg in range(2):
                    for par in range(2):
                        nc.tensor.transpose(
                            wt_ps[:, (2 * gg + par) * 128:
                                  (2 * gg + par) * 128 + 128],
                            wtile[:, gg * T + par * T2:
                                  gg * T + par * T2 + T2],
                            ident)
                wt_sb = w_pool.tile([T2, 512], BF16, tag="wt_sb")
                nc.vector.tensor_copy(out=wt_sb, in_=wt_ps)
                pp = ps_pool.tile([128, GW], F32, tag="pp")
                for gg in range(2):
                    g = 2 * g2 + gg
                    for j in range(4):
                        bb = 4 * g + j

                        def st32(par):
                            return wt_sb[:, (2 * gg + par) * 128 + 32 * j:
                                         (2 * gg + par) * 128 + 32 * j + 32]

                        nc.tensor.matmul(
                            pp[32 * j:32 * j + 32, D * gg:D * (gg + 1)],
                            st32(0), ntv[:, bb, 0, :],
                            start=True, stop=False,
                            skip_group_check=True,
                            tile_position=(0, 32 * j))
                        nc.tensor.matmul(
                            pp[32 * j:32 * j + 32, D * gg:D * (gg + 1)],
                            st32(1), ntv[:, bb, 1, :],
                            start=False, stop=True,
                            skip_group_check=True,
                            tile_position=(0, 32 * j))
                # wsum: column sums of wt_sb's replica columns, one col per
                # (gg, par, j); all 128 psum rows written via ones block
                wssl = wt_sb[:, 0:512]
                nc.tensor.matmul(
                    pp[:, 2 * D:GW],
                    ones128,
                    bass.AP(tensor=wssl.tensor, offset=wssl.offset,
                            ap=[wssl.ap[0], [32, 16]]),
                    start=True, stop=True, skip_group_check=True)
                if g2 % 2 == 0:
                    nc.vector.tensor_copy(
                        out=outt[:, GW * g2:GW * (g2 + 1)], in_=pp)
                else:
                    nc.scalar.activation(
                        outt[:, GW * g2:GW * (g2 + 1)], pp, AF.Copy)

            for j in range(4) if "pool" not in SKIP else []:
                nc.sync.dma_start(
                    out=out_dev.ap()[it, j, :],
                    in_=outt[32 * j:32 * j + 1, :],
                )

    return out_dev


def decode_out(arr, b_core=BC):
    """[nit, 4, OUTW] bf16 -> pooled [b_core, D], wsum [b_core]."""
    nit = b_core // B_IT
    a = np.asarray(arr).astype(np.float32).reshape(nit, 4, NGRP // 2, GW)
    p = a[..., 0:2 * D].reshape(nit, 4, NGRP // 2, 2, D)
    p = np.transpose(p, (0, 2, 3, 1, 4)).reshape(b_core, D)
    w = a[..., 2 * D:GW].reshape(nit, 4, NGRP // 2, 2, 2, 4)
    idx = np.arange(4)
    # rows 32j all hold the same sums; take row j for column j
    w = w[:, idx, :, :, :, idx]              # [4(j), nit, 8(g2), 2(gg), 2(par)]
    w = w.sum(axis=4)                        # sum parities
    w = np.transpose(w, (1, 2, 3, 0)).reshape(b_core)
    return p, w


_cache = {}
LAST_RESULT = None
SKIP = set()
CFG = dict(tp=2, mm=2, q=2, pool=2, ht=2, hh=1, outt=2, w=2)


def _get_program(b_core):
    key = (b_core, tuple(sorted(SKIP)), tuple(sorted(CFG.items())))
    if key not in _cache:
        nc = bacc.Bacc("TRN2", target_bir_lowering=False, debug=False,
                       num_devices=NCORES)
        build(nc, b_core)
        nc.compile()
        _cache[key] = nc
    return _cache[key]


def kernel(**inputs):
    hist = np.ascontiguousarray(np.asarray(inputs["hist_embeddings"], np.float32))
    tgt = np.ascontiguousarray(np.asarray(inputs["target_embedding"], np.float32))
    W = np.ascontiguousarray(np.asarray(inputs["W_kernel"], np.float32))
    Wb = np.ascontiguousarray(np.asarray(inputs["W_bias"], np.float32))
    q = np.ascontiguousarray(np.asarray(inputs["q_kernel"], np.float32))
    # q_bias shifts every logit equally -> softmax-invariant -> ignored.

    nc = _get_program(BC)
    in_maps = []
    for c in range(NCORES):
        sl = slice(c * BC, (c + 1) * BC)
        in_maps.append({
            "hist": hist[sl], "target": tgt[sl],
            "W": W, "Wb": Wb, "q": q,
        })
    res = run_bass_kernel_spmd(nc, in_maps, core_ids=list(range(NCORES)))
    global LAST_RESULT
    LAST_RESULT = res
    outs = []
    for c in range(NCORES):
        pooled, wsum = decode_out(res.results[c]["out_dev"])
        outs.append(pooled / wsum[:, None])
    return np.concatenate(outs, axis=0).astype(np.float32)


def timed_run(inputs, iters=5, bcs=BC):
    """Device-resident repeated execution; returns (best_seconds, outputs)."""
    import time
    import jax
    from jax.sharding import Mesh, PartitionSpec
    from jax.experimental.shard_map import shard_map
    import concourse.mybir as mybir_
    from concourse.bass2jax import (install_neuronx_cc_hook, _bass_exec_p,
                                    partition_id_tensor)

    hist = np.ascontiguousarray(np.asarray(inputs["hist_embeddings"], np.float32))
    tgt = np.ascontiguousarray(np.asarray(inputs["target_embedding"], np.float32))
    W = np.ascontiguousarray(np.asarray(inputs["W_kernel"], np.float32))
    Wb = np.ascontiguousarray(np.asarray(inputs["W_bias"], np.float32))
    q = np.ascontiguousarray(np.asarray(inputs["q_kernel"], np.float32))
    hist = hist[:NCORES * bcs]
    tgt = tgt[:NCORES * bcs]
    nc = _get_program(bcs)
    install_neuronx_cc_hook()

    pid_name = nc.partition_id_tensor.name if nc.partition_id_tensor else None
    in_names, out_names, out_avals, zero_outs = [], [], [], []
    for alloc in nc.m.functions[0].allocations:
        if not isinstance(alloc, mybir_.MemoryLocationSet):
            continue
        name = alloc.memorylocations[0].name
        if alloc.kind == "ExternalInput":
            if name != pid_name:
                in_names.append(name)
        elif alloc.kind == "ExternalOutput":
            shape = tuple(alloc.tensor_shape)
            dtype = mybir_.dt.np(alloc.dtype)
            out_names.append(name)
            out_avals.append(jax.core.ShapedArray(shape, dtype))
            zero_outs.append(np.zeros(shape, dtype))
    all_names = in_names + out_names
    if pid_name is not None:
        all_names = all_names + [pid_name]

    import os
    chain = int(os.environ.get("KERNEL_CHAIN", "1"))

    aliases = tuple((oi, len(in_names) + oi) for oi in range(len(out_names)))

    def _body(*args):
        nin_ = len(in_names)
        ins_ = list(args[:nin_])
        outs = list(args[nin_:])
        for _ in range(chain):
            operands = ins_ + outs
            if pid_name is not None:
                operands = operands + [partition_id_tensor()]
            outs = list(_bass_exec_p.bind(
                *operands, out_avals=tuple(out_avals),
                in_names=tuple(all_names), out_names=tuple(out_names),
                lowering_input_output_aliases=aliases,
                sim_require_finite=True, sim_require_nnan=True, nc=nc))
        return tuple(outs)

    devices = jax.devices()[:NCORES]
    mesh = Mesh(np.array(devices), ("core",))
    nin = len(in_names) + len(out_names)
    fn = jax.jit(shard_map(_body, mesh=mesh,
                           in_specs=(PartitionSpec("core"),) * nin,
                           out_specs=(PartitionSpec("core"),) * len(out_names),
                           check_rep=False),
                 donate_argnums=tuple(range(len(in_names), nin)))
    full = {"hist": hist, "target": tgt,
            "W": np.concatenate([W] * NCORES, 0),
            "Wb": np.concatenate([Wb] * NCORES, 0),
            "q": np.concatenate([q] * NCORES, 0)}
    args = [full[n] for n in in_names] + [
        np.concatenate([z] * NCORES, 0) for z in zero_outs]
    sh = jax.sharding.NamedSharding(mesh, PartitionSpec("core"))
    dargs = [jax.device_put(a, sh) for a in args]
    r = fn(*dargs)
    jax.block_until_ready(r)
    import os
    pipeline = int(os.environ.get("KERNEL_PIPE", "1"))
    nin_ = len(in_names)
    best = float("inf")
    for _ in range(iters):
        t0 = time.perf_counter()
        for _k in range(pipeline):
            r = fn(*dargs[:nin_], *r)
        jax.block_until_ready(r)
        best = min(best, time.perf_counter() - t0)
    outs = [np.asarray(x) for x in r]
    per_core = np.split(outs[out_names.index("out_dev")], NCORES, axis=0)
    full_out = []
    for c in range(NCORES):
        pooled, wsum = decode_out(per_core[c], bcs)
        full_out.append(pooled / wsum[:, None])
    return best, np.concatenate(full_out, 0).astype(np.float32)


if __name__ == "__main__":
    rng = np.random.default_rng(0)
    ins = {
        "target_embedding": rng.standard_normal((B, D), dtype=np.float32),
        "hist_embeddings": rng.standard_normal((B, T, D), dtype=np.float32),
        "W_kernel": (rng.standard_normal((D, D), dtype=np.float32) / np.sqrt(D)),
        "W_bias": np.zeros(D, np.float32),
        "q_kernel": (rng.standard_normal((D, 1), dtype=np.float32) / np.sqrt(D)),
        "q_bias": np.zeros(1, np.float32),
    }
    out = kernel(**ins)
    print("out", out.shape, out.dtype)

